# revision 1
# baseline (speedup 1.0000x reference)
"""Trainium2 Bass kernel for PersonalizedCalibrationNetwork (MoE-style judge routing).

Strategy: expert-parallel over the judge axis. Judge j lives on core j // 8.
The host routes samples to the core owning their judge, groups them by judge,
and pads every judge group to a uniform capacity C (so the single SPMD program
is shape-identical on all 8 cores). Each core computes, for its samples:

    z1 = sigmoid(x_aug @ (W1 + W1a[j]))      x_aug = [x, 1]
    z2 = sigmoid(z1_aug @ (W2 + W2a[j]))
    out = z2_aug @ (V + Va[j])               flattened to [257, 64]

All matmuls run transposed (features on partitions): z^T = G^T @ x^T, so layer
outputs feed the next layer without transposes. Per PSUM group (a bank-wide
strip of judge column-blocks):
  - the shared weight part is a full-width matmul,
  - each judge's weight part accumulates into its column slice,
  - all bias rows (8 judge + 1 shared) are applied by ONE K=9 matmul against a
    host-built 0/1 block mask [9, N].
Inputs are bf16 (fp32 accumulation in PSUM); host pre-packs every tensor in
the exact [128-partition, free] SBUF layout so every DMA is one contiguous
transfer, and the 6 input transfers are spread over 3 DGE rings (sync,
scalar, gpsimd) — each dma_start carries ~2us of completion latency, so few
big DMAs on parallel rings beat many small ones.
"""

import ml_dtypes
import numpy as np

import concourse.mybir as mybir
import concourse.tile as tile
from concourse.tile import add_dep_helper
from concourse import bacc
from concourse.bass_utils import run_bass_kernel_spmd


class _SlimTileContext(tile.TileContext):
    """TileContext with a slimmer kernel tail: one all-engine barrier
    before the semaphore clears instead of two (each engine simply halts
    after the clears; NRT waits for all engines before NEFF completion)."""

    def _drain_and_barrier(self, tick_clock, wait_clock):
        drain_inst = self.nc.sync.drain()
        wait_clock.add_sem_waits(
            drain_inst.ins, tile.ScopedClock({None: tick_clock.global_clock}))
        self.nc.all_engine_barrier()
        popped = self.nc._tile_sem_poison_stack.pop()
        assert popped is self._sem_poison
        self.nc.clear_and_free_semaphores(
            list(self.sems.allocated().values()))


N_CORES = 8
J = 64                 # judges
JPC = J // N_CORES     # judges per core
IN = 256               # input features (+1 bias)
L1 = 256
L2 = 256
Q = 16
A = 4
QA = Q * A             # 64 output columns
P = 128                # partitions
PSUM_W = 512           # fp32 psum bank width
KB = JPC + 1           # bias-matmul contraction dim (8 judge rows + shared)
NB = L1 + L2 + QA      # bias pack columns

BF16 = True
NP_W = ml_dtypes.bfloat16 if BF16 else np.float32

_cache = {}


def _make_groups(C):
    """Split the 8 judges into groups whose column strip fits a PSUM bank."""
    per_group = max(1, min(JPC, PSUM_W // C)) if C <= PSUM_W else 1
    groups = []  # (col0, gw, [(judge, ncol, width), ...])
    if C <= PSUM_W:
        for g0 in range(0, JPC, per_group):
            js = list(range(g0, min(g0 + per_group, JPC)))
            blocks = [(jj, jj * C, C) for jj in js]
            groups.append((g0 * C, len(js) * C, blocks))
    else:
        for jj in range(JPC):
            for c0 in range(0, C, PSUM_W):
                w = min(PSUM_W, C - c0)
                groups.append((jj * C + c0, w, [(jj, jj * C + c0, w)]))
    return groups


def _build_program(C):
    """Build + compile the SPMD Bass program for per-judge capacity C."""
    N = JPC * C  # padded samples per core
    groups = _make_groups(C)

    nc = bacc.Bacc("TRN2", target_bir_lowering=False, debug=False,
                   num_devices=N_CORES)
    dt = mybir.dt.bfloat16 if BF16 else mybir.dt.float32
    f32 = mybir.dt.float32

    # DRAM inputs, pre-packed host-side in SBUF layout
    xT_d = nc.dram_tensor("xT", [P, 2, N], dt, kind="ExternalInput")
    # shared weights [P, 2, 576]: cols [0:256) W1, [256:512) W2, [512:576) Vf
    wsh_d = nc.dram_tensor("wsh", [P, 2, NB], dt, kind="ExternalInput")
    w1a_d = nc.dram_tensor("w1a", [P, JPC, 2, L1], dt, kind="ExternalInput")
    w2a_d = nc.dram_tensor("w2a", [P, JPC, 2, L2], dt, kind="ExternalInput")
    va_d = nc.dram_tensor("va", [P, JPC, 2, QA], dt, kind="ExternalInput")
    # bias rows + block mask combined [KB, NB + N]:
    #   cols [0:NB) bias (rows 0..7 judge, row 8 shared),
    #   cols [NB:NB+N) mask (row jj = 1 on judge jj's columns, row 8 = ones)
    bm_d = nc.dram_tensor("bm", [KB, NB + N], dt, kind="ExternalInput")
    out_d = nc.dram_tensor("outT", [QA, N], f32, kind="ExternalOutput")

    with _SlimTileContext(nc) as tc:
        with (
            tc.tile_pool(name="const", bufs=1) as const,
            tc.tile_pool(name="psum", bufs=6, space="PSUM") as psum,
        ):
            xT = const.tile([P, 2, N], dt, tag="xT")
            wsh = const.tile([P, 2, NB], dt, tag="wsh")
            bm = const.tile([KB, NB + N], dt, tag="bm")
            w1a = const.tile([P, JPC, 2, L1], dt, tag="w1a")
            w2a = const.tile([P, JPC, 2, L2], dt, tag="w2a")
            va = const.tile([P, JPC, 2, QA], dt, tag="va")
            z1T = const.tile([P, 2, N], dt, tag="z1T")
            z2T = const.tile([P, 2, N], dt, tag="z2T")
            outT = const.tile([QA, N], f32, tag="outT")

            # Loads spread over 3 DGE rings in first-use order. The DMA
            # queues round-robin every outstanding transfer, so the later
            # weight loads are staggered behind w1a (via explicit deps) to
            # give the first-needed megabyte full bandwidth; layer-1
            # compute then overlaps the layer-2/3 weight transfers.
            nc.sync.dma_start(xT[:], xT_d[:])
            nc.scalar.dma_start(wsh[:], wsh_d[:])
            nc.gpsimd.dma_start(bm[:], bm_d[:])
            d_w1a = nc.sync.dma_start(w1a[:], w1a_d[:])
            d_w2a = nc.scalar.dma_start(w2a[:], w2a_d[:])
            d_va = nc.gpsimd.dma_start(va[:], va_d[:])
            add_dep_helper(d_w2a.ins, d_w1a.ins, reason="stagger w2a after w1a")
            add_dep_helper(d_va.ins, d_w1a.ins, reason="stagger va after w1a")

            def layer(sh_off, w_jd, rhs, M, zout):
                """z^T[M, N] = act(W^T @ rhs + b), accumulated per group."""
                n_m = (M + P - 1) // P
                for col0, gw, blocks in groups:
                    for m in range(n_m):
                        mw = min(P, M - m * P)
                        ps = psum.tile([P, PSUM_W], f32, tag="ps",
                                       name="ps")[:mw, :gw]
                        ms = slice(sh_off + m * P, sh_off + m * P + mw)
                        for ko in range(2):
                            nc.tensor.matmul(
                                ps, wsh[:, ko, ms],
                                rhs[:, ko, col0:col0 + gw],
                                start=(ko == 0), stop=False)
                        nc.tensor.matmul(
                            ps, bm[:, sh_off + m * P:sh_off + m * P + mw],
                            bm[:, NB + col0:NB + col0 + gw],
                            start=False, stop=False)
                        for bi, (jj, ncol, w) in enumerate(blocks):
                            off = ncol - col0
                            for ko in range(2):
                                nc.tensor.matmul(
                                    ps[:, off:off + w],
                                    w_jd[:, jj, ko, m * P:m * P + mw],
                                    rhs[:, ko, ncol:ncol + w],
                                    start=False,
                                    stop=(bi == len(blocks) - 1 and ko == 1))
                        if zout is not None:
                            nc.scalar.activation(
                                zout[:mw, m, col0:col0 + gw], ps,
                                mybir.ActivationFunctionType.Sigmoid)
                        else:
                            nc.vector.tensor_copy(
                                outT[:mw, col0:col0 + gw], ps)

            layer(0, w1a, xT, L1, z1T)
            layer(L1, w2a, z1T, L2, z2T)
            layer(L1 + L2, va, z2T, QA, None)

            nc.sync.dma_start(out_d[:], outT[:])

    nc.compile()
    return nc, N, groups


def kernel(X_machine_evals, X_human_judges, W1, W1a, W2, W2a, V, Va):
    X = np.asarray(X_machine_evals, dtype=np.float32)
    jid = np.asarray(X_human_judges).reshape(-1).astype(np.int64)
    W1 = np.asarray(W1, dtype=np.float32)
    W1a = np.asarray(W1a, dtype=np.float32)
    W2 = np.asarray(W2, dtype=np.float32)
    W2a = np.asarray(W2a, dtype=np.float32)
    V = np.asarray(V, dtype=np.float32)
    Va = np.asarray(Va, dtype=np.float32)
    B = X.shape[0]

    counts = np.bincount(jid, minlength=J)
    C = int(counts.max())

    if C not in _cache:
        _cache[C] = _build_program(C)
    nc, N, groups = _cache[C]

    # stable order of sample indices grouped by judge
    order = np.argsort(jid, kind="stable")
    sorted_j = jid[order]

    def pack_w(w):  # [256, M] -> [128, 2, M]
        M = w.shape[1]
        return np.ascontiguousarray(
            w[:256].reshape(2, P, M).transpose(1, 0, 2).astype(NP_W))

    Vf = V.transpose(1, 0, 2).reshape(IN + 1, QA)          # [257, 64]
    Vaf = Va.transpose(0, 2, 1, 3).reshape(J, IN + 1, QA)  # [J, 257, 64]

    wsh_in = np.ascontiguousarray(
        np.concatenate([pack_w(W1), pack_w(W2), pack_w(Vf)], axis=2))

    mask_in = np.zeros((KB, N), dtype=np.float32)
    mask_in[JPC, :] = 1
    for k in range(JPC):
        mask_in[k, k * C:(k + 1) * C] = 1

    in_maps = []
    core_meta = []
    for c in range(N_CORES):
        judges = np.arange(c * JPC, (c + 1) * JPC)
        Xp = np.zeros((N, IN), dtype=np.float32)
        samp = []  # per-judge sample indices
        for k, jj in enumerate(judges):
            idx = order[np.searchsorted(sorted_j, jj):
                        np.searchsorted(sorted_j, jj, side="right")]
            Xp[k * C:k * C + len(idx)] = X[idx]
            samp.append(idx)
        core_meta.append(samp)

        xT_in = np.ascontiguousarray(
            Xp.T.reshape(2, P, N).transpose(1, 0, 2).astype(NP_W))
        w1a_in = np.ascontiguousarray(
            np.stack([pack_w(W1a[jj]) for jj in judges], axis=1))
        w2a_in = np.ascontiguousarray(
            np.stack([pack_w(W2a[jj]) for jj in judges], axis=1))
        va_in = np.ascontiguousarray(
            np.stack([pack_w(Vaf[jj]) for jj in judges], axis=1))
        bm_in = np.empty((KB, NB + N), dtype=np.float32)
        bm_in[:JPC, :L1] = W1a[judges, 256]
        bm_in[:JPC, L1:L1 + L2] = W2a[judges, 256]
        bm_in[:JPC, L1 + L2:NB] = Vaf[judges, 256]
        bm_in[JPC, :NB] = np.concatenate([W1[256], W2[256], Vf[256]])
        bm_in[:, NB:] = mask_in
        in_maps.append({
            "xT": xT_in, "wsh": wsh_in,
            "w1a": w1a_in, "w2a": w2a_in, "va": va_in,
            "bm": bm_in.astype(NP_W),
        })

    res = run_bass_kernel_spmd(nc, in_maps, core_ids=list(range(N_CORES)))

    out = np.zeros((B, Q, A), dtype=np.float32)
    for c in range(N_CORES):
        oT = res.results[c]["outT"]          # [64, N]
        o = oT.T.reshape(N, Q, A)
        for k, idx in enumerate(core_meta[c]):
            out[idx] = o[k * C:k * C + len(idx)]
    return out



# revision 6
# speedup vs baseline: 1.1918x; 1.1918x over previous
"""Trainium2 Bass kernel for PersonalizedCalibrationNetwork (MoE-style judge routing).

Strategy: expert-parallel over the judge axis. Judge j lives on core j // 8.
The host routes samples to the core owning their judge, groups them by judge,
and pads every judge group to a uniform capacity C (so the single SPMD program
is shape-identical on all 8 cores). The host also folds the shared weights into
each judge's table (Wc[j] = W + Wa[j]) so the device does ONLY per-judge
matmuls — no separate shared-weight pass:

    z1 = sigmoid(x_aug @ W1c[j])      x_aug = [x, 1]
    z2 = sigmoid(z1_aug @ W2c[j])
    out = z2_aug @ Vc[j]              flattened to [257, 64]

All matmuls run transposed (features on partitions): z^T = Wc^T @ x^T, so layer
outputs feed the next layer without transposes. Per PSUM bank strip:
  - each judge's weights accumulate into its column slice (2 contraction
    chunks of 128); the very first matmul into a bank carries start=True
    (the PSUM zero-region clear is bank-wide, so exactly one start per bank),
  - all 8 judge bias rows are applied by ONE K=8 matmul against a host-built
    0/1 block mask [8, N].
The layer-1/2 judge tables are fp8e4m3 (stationary operand; moving stays bf16,
fp32 accumulation in PSUM) which halves the dominant HBM traffic; the output
table stays bf16 since fp8 there would quantize the logits directly.

Inputs stream over all three DGE rings concurrently (single-ring DGE tops out
near ~120 GB/s; HBM allows ~358) in first-use order, with the layer-1/2
tables split by output-feature half so compute chases the stream. While the
weights stream, throwaway matmuls on two alternating PSUM banks keep the PE
busy so the HAM clock-gate is warm (2.4 GHz, not 1.2) when real work arrives,
and a dummy activation hoists the sigmoid table loads off the critical path.
"""

import ml_dtypes
import numpy as np

import concourse.mybir as mybir
import concourse.tile as tile
from concourse import bacc
from concourse.bass_utils import run_bass_kernel_spmd


class _SlimTileContext(tile.TileContext):
    """TileContext with a slimmer kernel tail: one all-engine barrier
    before the semaphore clears instead of two (each engine simply halts
    after the clears; NRT waits for all engines before NEFF completion)."""

    def _drain_and_barrier(self, tick_clock, wait_clock):
        drain_inst = self.nc.sync.drain()
        wait_clock.add_sem_waits(
            drain_inst.ins, tile.ScopedClock({None: tick_clock.global_clock}))
        self.nc.all_engine_barrier()
        popped = self.nc._tile_sem_poison_stack.pop()
        assert popped is self._sem_poison
        self.nc.clear_and_free_semaphores(
            list(self.sems.allocated().values()))


N_CORES = 8
J = 64                 # judges
JPC = J // N_CORES     # judges per core
IN = 256               # input features (+1 bias)
L1 = 256
L2 = 256
Q = 16
A = 4
QA = Q * A             # 64 output columns
P = 128                # partitions
PSUM_W = 512           # fp32 psum bank width
KB = JPC               # bias-matmul contraction dim (8 judge rows)
NB = L1 + L2 + QA      # bias pack columns (576)

N_WARMUP = 8           # throwaway PE matmuls to warm the HAM clock gate

F8 = mybir.dt.float8e4
BF16 = mybir.dt.bfloat16
NP_F8 = mybir.dt.np(F8)
NP_BF16 = ml_dtypes.bfloat16

_cache = {}


def _make_groups(C):
    """Split the 8 judges into groups whose column strip fits a PSUM bank."""
    per_group = max(1, min(JPC, PSUM_W // C)) if C <= PSUM_W else 1
    groups = []  # (col0, gw, [(judge, ncol, width), ...])
    if C <= PSUM_W:
        for g0 in range(0, JPC, per_group):
            js = list(range(g0, min(g0 + per_group, JPC)))
            blocks = [(jj, jj * C, C) for jj in js]
            groups.append((g0 * C, len(js) * C, blocks))
    else:
        for jj in range(JPC):
            for c0 in range(0, C, PSUM_W):
                w = min(PSUM_W, C - c0)
                groups.append((jj * C + c0, w, [(jj, jj * C + c0, w)]))
    return groups


def _build_program(C):
    """Build + compile the SPMD Bass program for per-judge capacity C."""
    N = JPC * C  # padded samples per core
    groups = _make_groups(C)

    nc = bacc.Bacc("TRN2", target_bir_lowering=False, debug=False,
                   num_devices=N_CORES)
    f32 = mybir.dt.float32

    # DRAM inputs, pre-packed host-side in SBUF layout (combined weights).
    # w1/w2 carry the output-feature half (m) outermost in the free dims so
    # each half is one contiguous DMA.
    xT_d = nc.dram_tensor("xT", [P, 2, N], BF16, kind="ExternalInput")
    w1_d = nc.dram_tensor("w1", [P, 2, JPC, 2, P], F8, kind="ExternalInput")
    w2_d = nc.dram_tensor("w2", [P, 2, JPC, 2, P], F8, kind="ExternalInput")
    va_d = nc.dram_tensor("va", [P, JPC, 2, QA], BF16, kind="ExternalInput")
    # bias rows + block mask combined [KB, NB + N]:
    #   cols [0:NB) combined bias (row jj = judge jj, L1|L2|L3 packed),
    #   cols [NB:NB+N) mask (row jj = 1 on judge jj's columns)
    bm_d = nc.dram_tensor("bm", [KB, NB + N], BF16, kind="ExternalInput")
    out_d = nc.dram_tensor("outT", [QA, N], f32, kind="ExternalOutput")

    with _SlimTileContext(nc) as tc:
        with (
            tc.tile_pool(name="const", bufs=1) as const,
            tc.tile_pool(name="psum", bufs=6, space="PSUM") as psum,
            tc.tile_pool(name="psum_wu", bufs=2, space="PSUM") as psum_wu,
        ):
            xT = const.tile([P, 2, N], BF16, tag="xT")
            bm = const.tile([KB, NB + N], BF16, tag="bm")
            w1 = const.tile([P, 2, JPC, 2, P], F8, tag="w1")
            w2 = const.tile([P, 2, JPC, 2, P], F8, tag="w2")
            va = const.tile([P, JPC, 2, QA], BF16, tag="va")
            z1T = const.tile([P, 2, N], BF16, tag="z1T")
            z2T = const.tile([P, 2, N], BF16, tag="z2T")
            outT = const.tile([QA, N], f32, tag="outT")
            wu = const.tile([P, 640], BF16, tag="wu")       # warmup src
            fdum = const.tile([P, 8], f32, tag="fdum")      # act-table dummy
            bdum = const.tile([P, 8], BF16, tag="bdum")

            # Three rings streaming concurrently, each FIFO in first-use
            # order (a single DGE ring tops out around ~120 GB/s).
            nc.sync.dma_start(xT[:], xT_d[:])
            nc.sync.dma_start(w1[:, 0], w1_d[:, 0])
            nc.sync.dma_start(w1[:, 1], w1_d[:, 1])
            nc.scalar.dma_start(bm[:], bm_d[:])
            nc.scalar.dma_start(w2[:, 0], w2_d[:, 0])
            nc.scalar.dma_start(w2[:, 1], w2_d[:, 1])
            nc.gpsimd.dma_start(va[:], va_d[:])

            # Warm the PE (HAM clock gate needs ~3.4us of activity to lift
            # the 1.2GHz throttle) while the weights stream in, alternating
            # between two PSUM banks so the bank-wide zero-region clears
            # don't serialize back-to-back. A dummy activation pulls the
            # sigmoid table loads off the critical path.
            nc.vector.memset(wu[:], 0)
            nc.vector.memset(fdum[:], 0)
            nc.scalar.activation(bdum[:], fdum[:],
                                 mybir.ActivationFunctionType.Sigmoid)
            wu_ps = [psum_wu.tile([P, PSUM_W], f32, tag="wu", name="wu")
                     for _ in range(2)]
            for i in range(N_WARMUP):
                nc.tensor.matmul(wu_ps[i % 2][:], wu[:, :P],
                                 wu[:, P:P + PSUM_W],
                                 start=True, stop=True, skip_group_check=True)

            def layer(bias_off, w_jd, rhs, M, zout):
                """z^T[M, N] = act(Wc^T @ rhs + b), accumulated per group.

                w_jd indexing: callable (jj, ko, m, mw) -> lhsT AP [P, mw].
                """
                n_m = (M + P - 1) // P
                for col0, gw, blocks in groups:
                    for m in range(n_m):
                        mw = min(P, M - m * P)
                        ps = psum.tile([P, PSUM_W], f32, tag="ps",
                                       name="ps")[:mw, :gw]
                        first = True
                        for jj, ncol, w in blocks:
                            off = ncol - col0
                            for ko in range(2):
                                nc.tensor.matmul(
                                    ps[:, off:off + w],
                                    w_jd(jj, ko, m, mw),
                                    rhs[:, ko, ncol:ncol + w],
                                    start=first, stop=False)
                                first = False
                        if zout is not None:
                            nc.tensor.matmul(
                                ps, bm[:, bias_off + m * P:bias_off + m * P + mw],
                                bm[:, NB + col0:NB + col0 + gw],
                                start=False, stop=True)
                            nc.scalar.activation(
                                zout[:mw, m, col0:col0 + gw], ps,
                                mybir.ActivationFunctionType.Sigmoid)
                        else:
                            # output layer: finish + ship in two half-strips
                            # so the first out-DMA overlaps the second half.
                            # stop is sim-only bookkeeping; close the bank's
                            # group on the first half so its copy may read,
                            # and skip the checker for the second half.
                            halves = [(0, gw // 2, True), (gw // 2, gw, False)]
                            for h0, h1, st in halves:
                                nc.tensor.matmul(
                                    ps[:, h0:h1],
                                    bm[:, bias_off + m * P:bias_off + m * P + mw],
                                    bm[:, NB + col0 + h0:NB + col0 + h1],
                                    start=False, stop=st,
                                    skip_group_check=not st)
                                nc.vector.tensor_copy(
                                    outT[:mw, col0 + h0:col0 + h1],
                                    ps[:, h0:h1])
                                nc.sync.dma_start(
                                    out_d[:, col0 + h0:col0 + h1],
                                    outT[:, col0 + h0:col0 + h1])

            layer(0, lambda jj, ko, m, mw: w1[:, m, jj, ko, :mw],
                  xT, L1, z1T)
            layer(L1, lambda jj, ko, m, mw: w2[:, m, jj, ko, :mw],
                  z1T, L2, z2T)
            layer(L1 + L2, lambda jj, ko, m, mw: va[:, jj, ko, :mw],
                  z2T, QA, None)

    nc.compile()
    return nc, N, groups


def kernel(X_machine_evals, X_human_judges, W1, W1a, W2, W2a, V, Va):
    X = np.asarray(X_machine_evals, dtype=np.float32)
    jid = np.asarray(X_human_judges).reshape(-1).astype(np.int64)
    W1 = np.asarray(W1, dtype=np.float32)
    W1a = np.asarray(W1a, dtype=np.float32)
    W2 = np.asarray(W2, dtype=np.float32)
    W2a = np.asarray(W2a, dtype=np.float32)
    V = np.asarray(V, dtype=np.float32)
    Va = np.asarray(Va, dtype=np.float32)
    B = X.shape[0]

    counts = np.bincount(jid, minlength=J)
    C = int(counts.max())

    if C not in _cache:
        _cache[C] = _build_program(C)
    nc, N, groups = _cache[C]

    # stable order of sample indices grouped by judge
    order = np.argsort(jid, kind="stable")
    sorted_j = jid[order]

    def pack_w_m(w, np_dtype):  # [nj, 256, 256] -> [128, 2(m), nj, 2(ko), 128]
        nj = w.shape[0]
        return np.ascontiguousarray(
            w.reshape(nj, 2, P, 2, P).transpose(2, 3, 0, 1, 4).astype(np_dtype))

    def pack_w(w, np_dtype):  # [nj, 256, M] -> [128, nj, 2, M]
        nj, _, M = w.shape
        return np.ascontiguousarray(
            w.reshape(nj, 2, P, M).transpose(2, 0, 1, 3).astype(np_dtype))

    Vf = V.transpose(1, 0, 2).reshape(IN + 1, QA)          # [257, 64]
    Vaf = Va.transpose(0, 2, 1, 3).reshape(J, IN + 1, QA)  # [J, 257, 64]

    mask_in = np.zeros((KB, N), dtype=np.float32)
    for k in range(JPC):
        mask_in[k, k * C:(k + 1) * C] = 1

    in_maps = []
    core_meta = []
    for c in range(N_CORES):
        judges = np.arange(c * JPC, (c + 1) * JPC)
        w1c = W1[None] + W1a[judges]          # [8, 257, 256] combined
        w2c = W2[None] + W2a[judges]
        vac = Vf[None] + Vaf[judges]          # [8, 257, 64]

        Xp = np.zeros((N, IN), dtype=np.float32)
        samp = []  # per-judge sample indices
        for k, jj in enumerate(judges):
            idx = order[np.searchsorted(sorted_j, jj):
                        np.searchsorted(sorted_j, jj, side="right")]
            Xp[k * C:k * C + len(idx)] = X[idx]
            samp.append(idx)
        core_meta.append(samp)

        xT_in = np.ascontiguousarray(
            Xp.T.reshape(2, P, N).transpose(1, 0, 2).astype(NP_BF16))
        bm_in = np.empty((KB, NB + N), dtype=np.float32)
        bm_in[:, :L1] = w1c[:, 256]
        bm_in[:, L1:L1 + L2] = w2c[:, 256]
        bm_in[:, L1 + L2:NB] = vac[:, 256]
        bm_in[:, NB:] = mask_in
        in_maps.append({
            "xT": xT_in,
            "w1": pack_w_m(w1c[:, :256], NP_F8),
            "w2": pack_w_m(w2c[:, :256], NP_F8),
            "va": pack_w(vac[:, :256], NP_BF16),
            "bm": bm_in.astype(NP_BF16),
        })

    res = run_bass_kernel_spmd(nc, in_maps, core_ids=list(range(N_CORES)))

    out = np.zeros((B, Q, A), dtype=np.float32)
    for c in range(N_CORES):
        oT = res.results[c]["outT"]          # [64, N]
        o = oT.T.reshape(N, Q, A)
        for k, idx in enumerate(core_meta[c]):
            out[idx] = o[k * C:k * C + len(idx)]
    return out


# revision 10
# speedup vs baseline: 1.2179x; 1.0219x over previous
"""Trainium2 Bass kernel for PersonalizedCalibrationNetwork (MoE-style judge routing).

Strategy: expert-parallel over the judge axis. Judge j lives on core j // 8.
The host routes samples to the core owning their judge, groups them by judge,
and pads every judge group to a uniform capacity C (so the single SPMD program
is shape-identical on all 8 cores). The host also folds the shared weights into
each judge's table (Wc[j] = W + Wa[j]) so the device does ONLY per-judge
matmuls — no separate shared-weight pass:

    z1 = sigmoid(x_aug @ W1c[j])      x_aug = [x, 1]
    z2 = sigmoid(z1_aug @ W2c[j])
    out = z2_aug @ Vc[j]              flattened to [257, 64]

All matmuls run transposed (features on partitions): z^T = Wc^T @ x^T, so layer
outputs feed the next layer without transposes. Per PSUM bank strip:
  - each judge's weights accumulate into its column slice (2 contraction
    chunks of 128); the very first matmul into a bank carries start=True
    (the PSUM zero-region clear is bank-wide, so exactly one start per bank),
  - all 8 judge bias rows are applied by ONE K=8 matmul against a host-built
    0/1 block mask [8, N].
The layer-1/2 judge tables are fp8e4m3 (stationary operand; moving stays bf16,
fp32 accumulation in PSUM) which halves the dominant HBM traffic; the output
table stays bf16 since fp8 there would quantize the logits directly.

Inputs stream over all three DGE rings concurrently (single-ring DGE tops out
near ~120 GB/s; HBM allows ~358) in first-use order, with the layer-1/2
tables split by output-feature half so compute chases the stream. While the
weights stream, throwaway matmuls on two alternating PSUM banks keep the PE
busy so the HAM clock-gate is warm (2.4 GHz, not 1.2) when real work arrives,
and a dummy activation hoists the sigmoid table loads off the critical path.
"""

import ml_dtypes
import numpy as np

import concourse.mybir as mybir
import concourse.tile as tile
from concourse import bacc
from concourse.bass_utils import run_bass_kernel_spmd


class _SlimTileContext(tile.TileContext):
    """TileContext with a slimmer kernel tail: one all-engine barrier
    before the semaphore clears instead of two (each engine simply halts
    after the clears; NRT waits for all engines before NEFF completion)."""

    def _drain_and_barrier(self, tick_clock, wait_clock):
        drain_inst = self.nc.sync.drain()
        wait_clock.add_sem_waits(
            drain_inst.ins, tile.ScopedClock({None: tick_clock.global_clock}))
        self.nc.all_engine_barrier()
        popped = self.nc._tile_sem_poison_stack.pop()
        assert popped is self._sem_poison
        self.nc.clear_and_free_semaphores(
            list(self.sems.allocated().values()))


N_CORES = 8
J = 64                 # judges
JPC = J // N_CORES     # judges per core
IN = 256               # input features (+1 bias)
L1 = 256
L2 = 256
Q = 16
A = 4
QA = Q * A             # 64 output columns
P = 128                # partitions
PSUM_W = 512           # fp32 psum bank width
KB = JPC               # bias-matmul contraction dim (8 judge rows)
NB = L1 + L2 + QA      # bias pack columns (576)

N_WARMUP = 7           # throwaway PE matmuls to warm the HAM clock gate

F8 = mybir.dt.float8e4
BF16 = mybir.dt.bfloat16
NP_F8 = mybir.dt.np(F8)
NP_BF16 = ml_dtypes.bfloat16

_cache = {}


def _make_groups(C):
    """Split the 8 judges into groups whose column strip fits a PSUM bank."""
    per_group = max(1, min(JPC, PSUM_W // C)) if C <= PSUM_W else 1
    groups = []  # (col0, gw, [(judge, ncol, width), ...])
    if C <= PSUM_W:
        for g0 in range(0, JPC, per_group):
            js = list(range(g0, min(g0 + per_group, JPC)))
            blocks = [(jj, jj * C, C) for jj in js]
            groups.append((g0 * C, len(js) * C, blocks))
    else:
        for jj in range(JPC):
            for c0 in range(0, C, PSUM_W):
                w = min(PSUM_W, C - c0)
                groups.append((jj * C + c0, w, [(jj, jj * C + c0, w)]))
    return groups


def _build_program(C):
    """Build + compile the SPMD Bass program for per-judge capacity C."""
    N = JPC * C  # padded samples per core
    groups = _make_groups(C)

    nc = bacc.Bacc("TRN2", target_bir_lowering=False, debug=False,
                   num_devices=N_CORES)
    f32 = mybir.dt.float32

    # DRAM inputs, pre-packed host-side in SBUF layout (combined weights).
    # w1/w2 carry the output-feature half (m) outermost in the free dims so
    # each half is one contiguous DMA.
    xT_d = nc.dram_tensor("xT", [P, 2, N], BF16, kind="ExternalInput")
    w1_d = nc.dram_tensor("w1", [P, 2, JPC, 2, P], F8, kind="ExternalInput")
    w2_d = nc.dram_tensor("w2", [P, 2, JPC, 2, P], F8, kind="ExternalInput")
    va_d = nc.dram_tensor("va", [P, JPC, 2, QA], BF16, kind="ExternalInput")
    # bias rows + block mask combined [KB, NB + N]:
    #   cols [0:NB) combined bias (row jj = judge jj, L1|L2|L3 packed),
    #   cols [NB:NB+N) mask (row jj = 1 on judge jj's columns)
    bm_d = nc.dram_tensor("bm", [KB, NB + N], BF16, kind="ExternalInput")
    out_d = nc.dram_tensor("outT", [QA, N], f32, kind="ExternalOutput")

    with _SlimTileContext(nc) as tc:
        with (
            tc.tile_pool(name="const", bufs=1) as const,
            tc.tile_pool(name="psum", bufs=6, space="PSUM") as psum,
            tc.tile_pool(name="psum_wu", bufs=2, space="PSUM") as psum_wu,
        ):
            xT = const.tile([P, 2, N], BF16, tag="xT")
            bm = const.tile([KB, NB + N], BF16, tag="bm")
            w1 = const.tile([P, 2, JPC, 2, P], F8, tag="w1")
            w2 = const.tile([P, 2, JPC, 2, P], F8, tag="w2")
            va = const.tile([P, JPC, 2, QA], BF16, tag="va")
            z1T = const.tile([P, 2, N], BF16, tag="z1T")
            z2T = const.tile([P, 2, N], BF16, tag="z2T")
            outT = const.tile([QA, N], f32, tag="outT")
            wu = const.tile([P, 640], BF16, tag="wu")       # warmup src
            fdum = const.tile([P, 8], f32, tag="fdum")      # act-table dummy
            bdum = const.tile([P, 8], BF16, tag="bdum")

            # Three rings streaming concurrently, each FIFO in first-use
            # order. SWDGE (gpsimd) coalesces partition lines into bigger
            # descriptors and measures ~2x the per-ring rate of HWDGE, so
            # the first-needed big table goes there.
            nc.gpsimd.dma_start(w1[:], w1_d[:])
            nc.gpsimd.dma_start(va[:], va_d[:])
            nc.scalar.dma_start(bm[:], bm_d[:])
            nc.scalar.dma_start(w2[:], w2_d[:])
            nc.sync.dma_start(xT[:], xT_d[:])

            # Warm the PE (HAM clock gate needs ~3.4us of activity to lift
            # the 1.2GHz throttle) while the weights stream in, alternating
            # between two PSUM banks so the bank-wide zero-region clears
            # don't serialize back-to-back. A dummy activation pulls the
            # sigmoid table loads off the critical path.
            nc.vector.memset(wu[:], 0)
            nc.vector.memset(fdum[:], 0)
            nc.scalar.activation(bdum[:], fdum[:],
                                 mybir.ActivationFunctionType.Sigmoid)
            wu_ps = [psum_wu.tile([P, PSUM_W], f32, tag="wu", name="wu")
                     for _ in range(2)]
            for i in range(N_WARMUP):
                nc.tensor.matmul(wu_ps[i % 2][:], wu[:, :P],
                                 wu[:, P:P + PSUM_W],
                                 start=True, stop=True, skip_group_check=True)

            def layer(bias_off, w_jd, rhs, M, zout):
                """z^T[M, N] = act(Wc^T @ rhs + b), accumulated per group.

                w_jd indexing: callable (jj, ko, m, mw) -> lhsT AP [P, mw].
                """
                n_m = (M + P - 1) // P
                for col0, gw, blocks in groups:
                    for m in range(n_m):
                        mw = min(P, M - m * P)
                        ps = psum.tile([P, PSUM_W], f32, tag="ps",
                                       name="ps")[:mw, :gw]
                        first = True
                        for jj, ncol, w in blocks:
                            off = ncol - col0
                            for ko in range(2):
                                nc.tensor.matmul(
                                    ps[:, off:off + w],
                                    w_jd(jj, ko, m, mw),
                                    rhs[:, ko, ncol:ncol + w],
                                    start=first, stop=False)
                                first = False
                        if zout is not None:
                            nc.tensor.matmul(
                                ps, bm[:, bias_off + m * P:bias_off + m * P + mw],
                                bm[:, NB + col0:NB + col0 + gw],
                                start=False, stop=True)
                            # split the activation in column halves so the
                            # next layer's first matmuls start ~300ns sooner
                            for h0, h1 in ((0, gw // 2), (gw // 2, gw)):
                                nc.scalar.activation(
                                    zout[:mw, m, col0 + h0:col0 + h1],
                                    ps[:, h0:h1],
                                    mybir.ActivationFunctionType.Sigmoid)
                        else:
                            # output layer: finish + ship in two half-strips
                            # so the first out-DMA overlaps the second half.
                            # stop is sim-only bookkeeping; close the bank's
                            # group on the first half so its copy may read,
                            # and skip the checker for the second half. The
                            # two out-DMAs issue on different engines so
                            # their ~0.6us issue costs overlap.
                            halves = [(0, gw // 2, True, nc.sync),
                                      (gw // 2, gw, False, nc.scalar)]
                            for h0, h1, st, eng in halves:
                                nc.tensor.matmul(
                                    ps[:, h0:h1],
                                    bm[:, bias_off + m * P:bias_off + m * P + mw],
                                    bm[:, NB + col0 + h0:NB + col0 + h1],
                                    start=False, stop=st,
                                    skip_group_check=not st)
                                nc.vector.tensor_copy(
                                    outT[:mw, col0 + h0:col0 + h1],
                                    ps[:, h0:h1])
                                eng.dma_start(
                                    out_d[:, col0 + h0:col0 + h1],
                                    outT[:, col0 + h0:col0 + h1])

            layer(0, lambda jj, ko, m, mw: w1[:, m, jj, ko, :mw],
                  xT, L1, z1T)
            layer(L1, lambda jj, ko, m, mw: w2[:, m, jj, ko, :mw],
                  z1T, L2, z2T)
            layer(L1 + L2, lambda jj, ko, m, mw: va[:, jj, ko, :mw],
                  z2T, QA, None)

    nc.compile()
    return nc, N, groups


def kernel(X_machine_evals, X_human_judges, W1, W1a, W2, W2a, V, Va):
    X = np.asarray(X_machine_evals, dtype=np.float32)
    jid = np.asarray(X_human_judges).reshape(-1).astype(np.int64)
    W1 = np.asarray(W1, dtype=np.float32)
    W1a = np.asarray(W1a, dtype=np.float32)
    W2 = np.asarray(W2, dtype=np.float32)
    W2a = np.asarray(W2a, dtype=np.float32)
    V = np.asarray(V, dtype=np.float32)
    Va = np.asarray(Va, dtype=np.float32)
    B = X.shape[0]

    counts = np.bincount(jid, minlength=J)
    C = int(counts.max())

    if C not in _cache:
        _cache[C] = _build_program(C)
    nc, N, groups = _cache[C]

    # stable order of sample indices grouped by judge
    order = np.argsort(jid, kind="stable")
    sorted_j = jid[order]

    def pack_w_m(w, np_dtype):  # [nj, 256, 256] -> [128, 2(m), nj, 2(ko), 128]
        nj = w.shape[0]
        return np.ascontiguousarray(
            w.reshape(nj, 2, P, 2, P).transpose(2, 3, 0, 1, 4).astype(np_dtype))

    def pack_w(w, np_dtype):  # [nj, 256, M] -> [128, nj, 2, M]
        nj, _, M = w.shape
        return np.ascontiguousarray(
            w.reshape(nj, 2, P, M).transpose(2, 0, 1, 3).astype(np_dtype))

    Vf = V.transpose(1, 0, 2).reshape(IN + 1, QA)          # [257, 64]
    Vaf = Va.transpose(0, 2, 1, 3).reshape(J, IN + 1, QA)  # [J, 257, 64]

    mask_in = np.zeros((KB, N), dtype=np.float32)
    for k in range(JPC):
        mask_in[k, k * C:(k + 1) * C] = 1

    in_maps = []
    core_meta = []
    for c in range(N_CORES):
        judges = np.arange(c * JPC, (c + 1) * JPC)
        w1c = W1[None] + W1a[judges]          # [8, 257, 256] combined
        w2c = W2[None] + W2a[judges]
        vac = Vf[None] + Vaf[judges]          # [8, 257, 64]

        Xp = np.zeros((N, IN), dtype=np.float32)
        samp = []  # per-judge sample indices
        for k, jj in enumerate(judges):
            idx = order[np.searchsorted(sorted_j, jj):
                        np.searchsorted(sorted_j, jj, side="right")]
            Xp[k * C:k * C + len(idx)] = X[idx]
            samp.append(idx)
        core_meta.append(samp)

        xT_in = np.ascontiguousarray(
            Xp.T.reshape(2, P, N).transpose(1, 0, 2).astype(NP_BF16))
        bm_in = np.empty((KB, NB + N), dtype=np.float32)
        bm_in[:, :L1] = w1c[:, 256]
        bm_in[:, L1:L1 + L2] = w2c[:, 256]
        bm_in[:, L1 + L2:NB] = vac[:, 256]
        bm_in[:, NB:] = mask_in
        in_maps.append({
            "xT": xT_in,
            "w1": pack_w_m(w1c[:, :256], NP_F8),
            "w2": pack_w_m(w2c[:, :256], NP_F8),
            "va": pack_w(vac[:, :256], NP_BF16),
            "bm": bm_in.astype(NP_BF16),
        })

    res = run_bass_kernel_spmd(nc, in_maps, core_ids=list(range(N_CORES)))

    out = np.zeros((B, Q, A), dtype=np.float32)
    for c in range(N_CORES):
        oT = res.results[c]["outT"]          # [64, N]
        o = oT.T.reshape(N, Q, A)
        for k, idx in enumerate(core_meta[c]):
            out[idx] = o[k * C:k * C + len(idx)]
    return out


# revision 11
# speedup vs baseline: 1.2219x; 1.0033x over previous
"""Trainium2 Bass kernel for PersonalizedCalibrationNetwork (MoE-style judge routing).

Strategy: expert-parallel over the judge axis. Judge j lives on core j // 8.
The host routes samples to the core owning their judge, groups them by judge,
and pads every judge group to a uniform capacity C (so the single SPMD program
is shape-identical on all 8 cores). The host also folds the shared weights into
each judge's table (Wc[j] = W + Wa[j]) so the device does ONLY per-judge
matmuls — no separate shared-weight pass:

    z1 = sigmoid(x_aug @ W1c[j])      x_aug = [x, 1]
    z2 = sigmoid(z1_aug @ W2c[j])
    out = z2_aug @ Vc[j]              flattened to [257, 64]

All matmuls run transposed (features on partitions): z^T = Wc^T @ x^T, so layer
outputs feed the next layer without transposes. Per PSUM bank strip:
  - each judge's weights accumulate into its column slice (2 contraction
    chunks of 128); the very first matmul into a bank carries start=True
    (the PSUM zero-region clear is bank-wide, so exactly one start per bank),
  - all 8 judge bias rows are applied by ONE K=8 matmul against a host-built
    0/1 block mask [8, N].
The layer-1/2 judge tables are fp8e4m3 (stationary operand; moving stays bf16,
fp32 accumulation in PSUM) which halves the dominant HBM traffic; the output
table stays bf16 since fp8 there would quantize the logits directly.

Inputs stream over all three DGE rings concurrently (single-ring DGE tops out
near ~120 GB/s; HBM allows ~358) in first-use order, with the layer-1/2
tables split by output-feature half so compute chases the stream. While the
weights stream, throwaway matmuls on two alternating PSUM banks keep the PE
busy so the HAM clock-gate is warm (2.4 GHz, not 1.2) when real work arrives,
and a dummy activation hoists the sigmoid table loads off the critical path.
"""

import ml_dtypes
import numpy as np

import concourse.mybir as mybir
import concourse.tile as tile
from concourse import bacc
from concourse.bass_utils import run_bass_kernel_spmd


class _SlimTileContext(tile.TileContext):
    """TileContext with a slimmer kernel tail: one all-engine barrier
    before the semaphore clears instead of two (each engine simply halts
    after the clears; NRT waits for all engines before NEFF completion)."""

    def _drain_and_barrier(self, tick_clock, wait_clock):
        drain_inst = self.nc.sync.drain()
        wait_clock.add_sem_waits(
            drain_inst.ins, tile.ScopedClock({None: tick_clock.global_clock}))
        self.nc.all_engine_barrier()
        popped = self.nc._tile_sem_poison_stack.pop()
        assert popped is self._sem_poison
        self.nc.clear_and_free_semaphores(
            list(self.sems.allocated().values()))


N_CORES = 8
J = 64                 # judges
JPC = J // N_CORES     # judges per core
IN = 256               # input features (+1 bias)
L1 = 256
L2 = 256
Q = 16
A = 4
QA = Q * A             # 64 output columns
P = 128                # partitions
PSUM_W = 512           # fp32 psum bank width
KB = JPC               # bias-matmul contraction dim (8 judge rows)
NB = L1 + L2 + QA      # bias pack columns (576)

N_WARMUP = 6           # throwaway PE matmuls to warm the HAM clock gate

F8 = mybir.dt.float8e4
BF16 = mybir.dt.bfloat16
NP_F8 = mybir.dt.np(F8)
NP_BF16 = ml_dtypes.bfloat16

_cache = {}


def _make_groups(C):
    """Split the 8 judges into groups whose column strip fits a PSUM bank."""
    per_group = max(1, min(JPC, PSUM_W // C)) if C <= PSUM_W else 1
    groups = []  # (col0, gw, [(judge, ncol, width), ...])
    if C <= PSUM_W:
        for g0 in range(0, JPC, per_group):
            js = list(range(g0, min(g0 + per_group, JPC)))
            blocks = [(jj, jj * C, C) for jj in js]
            groups.append((g0 * C, len(js) * C, blocks))
    else:
        for jj in range(JPC):
            for c0 in range(0, C, PSUM_W):
                w = min(PSUM_W, C - c0)
                groups.append((jj * C + c0, w, [(jj, jj * C + c0, w)]))
    return groups


def _build_program(C):
    """Build + compile the SPMD Bass program for per-judge capacity C."""
    N = JPC * C  # padded samples per core
    groups = _make_groups(C)

    nc = bacc.Bacc("TRN2", target_bir_lowering=False, debug=False,
                   num_devices=N_CORES)
    f32 = mybir.dt.float32

    # DRAM inputs, pre-packed host-side in SBUF layout (combined weights).
    # w1/w2 carry the output-feature half (m) outermost in the free dims so
    # each half is one contiguous DMA.
    xT_d = nc.dram_tensor("xT", [P, 2, N], F8, kind="ExternalInput")
    w1_d = nc.dram_tensor("w1", [P, 2, JPC, 2, P], F8, kind="ExternalInput")
    w2_d = nc.dram_tensor("w2", [P, 2, JPC, 2, P], F8, kind="ExternalInput")
    va_d = nc.dram_tensor("va", [P, JPC, 2, QA], BF16, kind="ExternalInput")
    # bias rows + block mask combined [KB, NB + N]:
    #   cols [0:NB) combined bias (row jj = judge jj, L1|L2|L3 packed),
    #   cols [NB:NB+N) mask (row jj = 1 on judge jj's columns)
    bm_d = nc.dram_tensor("bm", [KB, NB + N], BF16, kind="ExternalInput")
    out_d = nc.dram_tensor("outT", [QA, N], f32, kind="ExternalOutput")

    with _SlimTileContext(nc) as tc:
        with (
            tc.tile_pool(name="const", bufs=1) as const,
            tc.tile_pool(name="psum", bufs=6, space="PSUM") as psum,
            tc.tile_pool(name="psum_wu", bufs=2, space="PSUM") as psum_wu,
        ):
            xT = const.tile([P, 2, N], F8, tag="xT")
            bm = const.tile([KB, NB + N], BF16, tag="bm")
            w1 = const.tile([P, 2, JPC, 2, P], F8, tag="w1")
            w2 = const.tile([P, 2, JPC, 2, P], F8, tag="w2")
            va = const.tile([P, JPC, 2, QA], BF16, tag="va")
            z1T = const.tile([P, 2, N], BF16, tag="z1T")
            z2T = const.tile([P, 2, N], BF16, tag="z2T")
            outT = const.tile([QA, N], f32, tag="outT")
            wu = const.tile([P, 640], BF16, tag="wu")       # warmup src
            fdum = const.tile([P, 8], f32, tag="fdum")      # act-table dummy
            bdum = const.tile([P, 8], BF16, tag="bdum")

            # Three rings streaming concurrently, each FIFO in first-use
            # order, bytes balanced so no ring drags the tail. SWDGE
            # (gpsimd) coalesces partition lines into bigger descriptors
            # and measures ~2x the per-ring rate of HWDGE, so the
            # first-needed big table goes there, split by output half so
            # compute chases the stream.
            nc.gpsimd.dma_start(w1[:, 0], w1_d[:, 0])
            nc.gpsimd.dma_start(w1[:, 1], w1_d[:, 1])
            nc.gpsimd.dma_start(va[:], va_d[:])
            nc.sync.dma_start(xT[:], xT_d[:])
            nc.sync.dma_start(w2[:, 0], w2_d[:, 0])
            nc.scalar.dma_start(bm[:], bm_d[:])
            nc.scalar.dma_start(w2[:, 1], w2_d[:, 1])

            # Warm the PE (HAM clock gate needs ~3.4us of activity to lift
            # the 1.2GHz throttle) while the weights stream in, alternating
            # between two PSUM banks so the bank-wide zero-region clears
            # don't serialize back-to-back. A dummy activation pulls the
            # sigmoid table loads off the critical path.
            nc.vector.memset(wu[:], 0)
            nc.vector.memset(fdum[:], 0)
            nc.scalar.activation(bdum[:], fdum[:],
                                 mybir.ActivationFunctionType.Sigmoid)
            wu_ps = [psum_wu.tile([P, PSUM_W], f32, tag="wu", name="wu")
                     for _ in range(2)]
            for i in range(N_WARMUP):
                nc.tensor.matmul(wu_ps[i % 2][:], wu[:, :P],
                                 wu[:, P:P + PSUM_W],
                                 start=True, stop=True, skip_group_check=True)

            def layer(bias_off, w_jd, rhs, M, zout):
                """z^T[M, N] = act(Wc^T @ rhs + b), accumulated per group.

                The bias+mask matmul goes FIRST into each bank (its operand
                bm lands early, and start=True clears the bank anyway), so
                the layer boundary is just last-judge-matmul -> activation.
                Judges run ko-outer so the ko=1 chunk chases the previous
                layer's second-half activations.

                w_jd indexing: callable (jj, ko, m, mw) -> lhsT AP [P, mw].
                """
                n_m = (M + P - 1) // P
                for col0, gw, blocks in groups:
                    for m in range(n_m):
                        mw = min(P, M - m * P)
                        ps = psum.tile([P, PSUM_W], f32, tag="ps",
                                       name="ps")[:mw, :gw]
                        if zout is not None:
                            nc.tensor.matmul(
                                ps, bm[:, bias_off + m * P:bias_off + m * P + mw],
                                bm[:, NB + col0:NB + col0 + gw],
                                start=True, stop=False)
                            for ko in range(2):
                                for bi, (jj, ncol, w) in enumerate(blocks):
                                    off = ncol - col0
                                    nc.tensor.matmul(
                                        ps[:, off:off + w],
                                        w_jd(jj, ko, m, mw),
                                        rhs[:, ko, ncol:ncol + w],
                                        start=False,
                                        stop=(ko == 1
                                              and bi == len(blocks) - 1))
                            # split the activation in column halves so the
                            # next layer's first matmuls start sooner
                            for h0, h1 in ((0, gw // 2), (gw // 2, gw)):
                                nc.scalar.activation(
                                    zout[:mw, m, col0 + h0:col0 + h1],
                                    ps[:, h0:h1],
                                    mybir.ActivationFunctionType.Sigmoid)
                        else:
                            # output layer: bias halves first, then judges
                            # ko-outer; ship each half-strip as soon as its
                            # last matmul lands so the first out-DMA
                            # overlaps the rest of the compute. stop is
                            # sim-only bookkeeping: close the bank group
                            # with the last matmul of the FIRST half and
                            # skip the checker for the trailing ones.
                            hm = gw // 2
                            nc.tensor.matmul(
                                ps[:, :hm],
                                bm[:, bias_off + m * P:bias_off + m * P + mw],
                                bm[:, NB + col0:NB + col0 + hm],
                                start=True, stop=False)
                            nc.tensor.matmul(
                                ps[:, hm:],
                                bm[:, bias_off + m * P:bias_off + m * P + mw],
                                bm[:, NB + col0 + hm:NB + col0 + gw],
                                start=False, stop=False)
                            nb = len(blocks)
                            for ko in range(2):
                                for bi, (jj, ncol, w) in enumerate(blocks):
                                    off = ncol - col0
                                    closing = (ko == 1 and bi == nb // 2 - 1)
                                    trailing = (ko == 1 and bi >= nb // 2)
                                    nc.tensor.matmul(
                                        ps[:, off:off + w],
                                        w_jd(jj, ko, m, mw),
                                        rhs[:, ko, ncol:ncol + w],
                                        start=False, stop=closing,
                                        skip_group_check=trailing)
                                    if closing:
                                        nc.vector.tensor_copy(
                                            outT[:mw, col0:col0 + hm],
                                            ps[:, :hm])
                                        nc.sync.dma_start(
                                            out_d[:, col0:col0 + hm],
                                            outT[:, col0:col0 + hm])
                            nc.vector.tensor_copy(
                                outT[:mw, col0 + hm:col0 + gw], ps[:, hm:])
                            nc.scalar.dma_start(
                                out_d[:, col0 + hm:col0 + gw],
                                outT[:, col0 + hm:col0 + gw])

            layer(0, lambda jj, ko, m, mw: w1[:, m, jj, ko, :mw],
                  xT, L1, z1T)
            layer(L1, lambda jj, ko, m, mw: w2[:, m, jj, ko, :mw],
                  z1T, L2, z2T)
            layer(L1 + L2, lambda jj, ko, m, mw: va[:, jj, ko, :mw],
                  z2T, QA, None)

    nc.compile()
    return nc, N, groups


def kernel(X_machine_evals, X_human_judges, W1, W1a, W2, W2a, V, Va):
    X = np.asarray(X_machine_evals, dtype=np.float32)
    jid = np.asarray(X_human_judges).reshape(-1).astype(np.int64)
    W1 = np.asarray(W1, dtype=np.float32)
    W1a = np.asarray(W1a, dtype=np.float32)
    W2 = np.asarray(W2, dtype=np.float32)
    W2a = np.asarray(W2a, dtype=np.float32)
    V = np.asarray(V, dtype=np.float32)
    Va = np.asarray(Va, dtype=np.float32)
    B = X.shape[0]

    counts = np.bincount(jid, minlength=J)
    C = int(counts.max())

    if C not in _cache:
        _cache[C] = _build_program(C)
    nc, N, groups = _cache[C]

    # stable order of sample indices grouped by judge
    order = np.argsort(jid, kind="stable")
    sorted_j = jid[order]

    def pack_w_m(w, np_dtype):  # [nj, 256, 256] -> [128, 2(m), nj, 2(ko), 128]
        nj = w.shape[0]
        return np.ascontiguousarray(
            w.reshape(nj, 2, P, 2, P).transpose(2, 3, 0, 1, 4).astype(np_dtype))

    def pack_w(w, np_dtype):  # [nj, 256, M] -> [128, nj, 2, M]
        nj, _, M = w.shape
        return np.ascontiguousarray(
            w.reshape(nj, 2, P, M).transpose(2, 0, 1, 3).astype(np_dtype))

    Vf = V.transpose(1, 0, 2).reshape(IN + 1, QA)          # [257, 64]
    Vaf = Va.transpose(0, 2, 1, 3).reshape(J, IN + 1, QA)  # [J, 257, 64]

    mask_in = np.zeros((KB, N), dtype=np.float32)
    for k in range(JPC):
        mask_in[k, k * C:(k + 1) * C] = 1

    in_maps = []
    core_meta = []
    for c in range(N_CORES):
        judges = np.arange(c * JPC, (c + 1) * JPC)
        w1c = W1[None] + W1a[judges]          # [8, 257, 256] combined
        w2c = W2[None] + W2a[judges]
        vac = Vf[None] + Vaf[judges]          # [8, 257, 64]

        Xp = np.zeros((N, IN), dtype=np.float32)
        samp = []  # per-judge sample indices
        for k, jj in enumerate(judges):
            idx = order[np.searchsorted(sorted_j, jj):
                        np.searchsorted(sorted_j, jj, side="right")]
            Xp[k * C:k * C + len(idx)] = X[idx]
            samp.append(idx)
        core_meta.append(samp)

        xT_in = np.ascontiguousarray(
            Xp.T.reshape(2, P, N).transpose(1, 0, 2).astype(NP_F8))
        bm_in = np.empty((KB, NB + N), dtype=np.float32)
        bm_in[:, :L1] = w1c[:, 256]
        bm_in[:, L1:L1 + L2] = w2c[:, 256]
        bm_in[:, L1 + L2:NB] = vac[:, 256]
        bm_in[:, NB:] = mask_in
        in_maps.append({
            "xT": xT_in,
            "w1": pack_w_m(w1c[:, :256], NP_F8),
            "w2": pack_w_m(w2c[:, :256], NP_F8),
            "va": pack_w(vac[:, :256], NP_BF16),
            "bm": bm_in.astype(NP_BF16),
        })

    res = run_bass_kernel_spmd(nc, in_maps, core_ids=list(range(N_CORES)))

    out = np.zeros((B, Q, A), dtype=np.float32)
    for c in range(N_CORES):
        oT = res.results[c]["outT"]          # [64, N]
        o = oT.T.reshape(N, Q, A)
        for k, idx in enumerate(core_meta[c]):
            out[idx] = o[k * C:k * C + len(idx)]
    return out
